# revision 29
# baseline (speedup 1.0000x reference)
"""Trainium2 Bass kernel for nn_NodeAttention (gnn_message_passing), v2.

Strategy (8 cores, data-parallel over nodes; weights + x_1 replicated):

Phase A (per core): build bf16 table T[n] = [RoPE(x_1@Wk) | x_1@Wv] for all
  20480 (padded) nodes. Host supplies x_1 pre-transposed (feature-major) so
  the stationary loads need no on-chip transpose; 4-tile-batched DMAs; RoPE
  as 3 DVE ops using a phase-shifted sin table (one Sin activation yields
  [cos | signed-sin]); V copied out of PSUM on Act/Pool.

Phase C (per core, 20 tiles of its padded 2560-node shard): per tile
  - 16 indirect row gathers (neighbor K|V rows, 1KB each) on the SWDGE
  - q/gate matmuls (stationary = host-transposed x_1 slice)
  - bias2 = LN(x_2)@Wb via algebraic refactor:
      rstd*(x2@(g*Wb) - mean*(g@Wb)) + b@Wb
    with mean as an extra matmul column (node-stationary, using host
    feature-major x_2) and sum-of-squares via a ones-stationary matmul on
    x_2^2 whose [1,2048] transposed result is reshaped node-major by a
    small SBUF->SBUF DMA.
  - rstd via bit-trick + Newton rsqrt on DVE (no Sqrt activation table)
  - sigmoid gate via tanh (same activation table as Exp)
  - scores/softmax/weighted-V elementwise on DVE, output matmul, final LN.

Activation tables: phase A uses only Sin/Copy, phase C only
Exp/Tanh/Square/Copy/Identity -> exactly two table loads.
"""
import sys, math
if "/opt/trn_rl_repo" not in sys.path:
    sys.path.insert(0, "/opt/trn_rl_repo")

import numpy as np
import ml_dtypes
from contextlib import ExitStack

import concourse.bass as bass
import concourse.tile as tile
from concourse import bacc, mybir
from concourse.bass import IndirectOffsetOnAxis
from concourse.bass_utils import run_bass_kernel_spmd

P = 128
KZ, IFZ, AHZ, AFZ = 16, 256, 8, 32
HF = AHZ * AFZ          # 256
EPS = 1e-5
F32 = mybir.dt.float32
BF16 = mybir.dt.bfloat16
I32 = mybir.dt.int32
AF = mybir.ActivationFunctionType
OP = mybir.AluOpType
N_CORES = 8
N_FULL = 20000
NP = 20480              # padded table rows (160 tiles)
NT1 = NP // P           # 160
CH = 4                  # phase-A tiles per DMA chunk
NCH = NT1 // CH         # 40
NSH = 2560              # padded shard rows (20 tiles)
NT2 = NSH // P          # 20

BF = ml_dtypes.bfloat16
MAGIC = 0x5F3759DF


def _newton_rsqrt(nc, pool, v_ap, n_free, tag):
    """rstd = 1/sqrt(v) on DVE via bit-trick seed + 2 Newton iterations.
    v_ap: [P, n_free] f32 AP (must be a plain SBUF tile view)."""
    ti = pool.tile([P, n_free], I32, tag=f"{tag}_i")
    nc.vector.tensor_scalar(ti[:], v_ap.bitcast(I32), 1, None,
                            op0=OP.logical_shift_right)
    nc.vector.tensor_scalar(ti[:], ti[:], -1, MAGIC, op0=OP.mult, op1=OP.add)
    y = pool.tile([P, n_free], F32, tag=f"{tag}_y")
    t2 = pool.tile([P, n_free], F32, tag=f"{tag}_t")
    yf = ti[:].bitcast(F32)
    nc.vector.tensor_tensor(t2[:], yf, yf, op=OP.mult)
    nc.vector.tensor_tensor(t2[:], t2[:], v_ap, op=OP.mult)
    nc.vector.tensor_scalar(t2[:], t2[:], -0.5, 1.5, op0=OP.mult, op1=OP.add)
    nc.vector.tensor_tensor(y[:], yf, t2[:], op=OP.mult)
    nc.vector.tensor_tensor(t2[:], y[:], y[:], op=OP.mult)
    nc.vector.tensor_tensor(t2[:], t2[:], v_ap, op=OP.mult)
    nc.vector.tensor_scalar(t2[:], t2[:], -0.5, 1.5, op0=OP.mult, op1=OP.add)
    nc.vector.tensor_tensor(y[:], y[:], t2[:], op=OP.mult)
    return y


def build_nc(n_cores=N_CORES):
    nc = bacc.Bacc("TRN2", target_bir_lowering=False, debug=False,
                   num_devices=n_cores)

    # ---------------- dram I/O ----------------
    x1t = nc.dram_tensor("x1t", [2, P, NP], BF16, kind="ExternalInput")
    x1qo = nc.dram_tensor("x1qo", [P, 2, NSH], BF16, kind="ExternalInput")
    posf = nc.dram_tensor("posf", [P, NT1, 2 * AFZ], F32, kind="ExternalInput")
    poso = nc.dram_tensor("poso", [P, NT2, 2 * AFZ], F32, kind="ExternalInput")
    x2t = nc.dram_tensor("x2t", [P, 2, NT2, KZ, P], BF16, kind="ExternalInput")
    eit = nc.dram_tensor("eit", [P, NT2, KZ], I32, kind="ExternalInput")
    x1rt = nc.dram_tensor("x1rt", [P, NT2, IFZ], F32, kind="ExternalInput")
    wkv = nc.dram_tensor("wkv", [P, 2, 2 * HF], BF16, kind="ExternalInput")
    wqg = nc.dram_tensor("wqg", [P, 2, 2 * HF], BF16, kind="ExternalInput")
    wb16 = nc.dram_tensor("wb16", [P, 2, 16], BF16, kind="ExternalInput")
    wback = nc.dram_tensor("wback", [P, 2, IFZ], BF16, kind="ExternalInput")
    vecs = nc.dram_tensor("vecs", [1, 5 * IFZ + 16], F32, kind="ExternalInput")
    # vecs layout: [bg(256) | lng(256) | lnb(256) | bback(256) | x?256 unused |
    #               sg8(8) tb8(8)]
    out = nc.dram_tensor("out", [NSH, IFZ], F32, kind="ExternalOutput")

    with tile.TileContext(nc) as tc, ExitStack() as ctx:
        const = ctx.enter_context(tc.tile_pool(name="const", bufs=1))
        dram = ctx.enter_context(tc.tile_pool(name="dram", bufs=1, space="DRAM"))

        # ---------------- constants ----------------
        wkvb = const.tile([P, 2, 2 * HF], BF16)
        nc.scalar.dma_start(wkvb[:], wkv[:, :, :])
        wqgb = const.tile([P, 2, 2 * HF], BF16)
        nc.scalar.dma_start(wqgb[:], wqg[:, :, :])
        wbb = const.tile([P, 2, 16], BF16)
        nc.scalar.dma_start(wbb[:], wb16[:, :, :])
        wbackb = const.tile([P, 2, IFZ], BF16)
        nc.scalar.dma_start(wbackb[:], wback[:, :, :])
        vec_r = const.tile([P, 5 * IFZ + 16], F32)
        nc.scalar.dma_start(vec_r[:], vecs[0:1, :].to_broadcast(
            [P, 5 * IFZ + 16]))
        bg_r = vec_r[:, 0:IFZ]
        lng_r = vec_r[:, IFZ:2 * IFZ]
        lnb_r = vec_r[:, 2 * IFZ:3 * IFZ]
        bback_r = vec_r[:, 3 * IFZ:4 * IFZ]
        sg_r = vec_r[:, 5 * IFZ:5 * IFZ + 8]
        tb_r = vec_r[:, 5 * IFZ + 8:5 * IFZ + 16]
        ones1 = const.tile([P, 1], BF16)
        nc.gpsimd.memset(ones1[:], 1.0)

        Tt = dram.tile([NP, 2 * HF], BF16)

        # =============== phase A: build K|V table ===============
        with tc.tile_pool(name="apool", bufs=3) as ap, \
             tc.tile_pool(name="apsum", bufs=3, space="PSUM") as aps:
            for cc in range(NCH):
                x1c = ap.tile([P, 2, CH * P], BF16)
                nc.sync.dma_start(
                    x1c[:], x1t[:, :, cc * CH * P:(cc + 1) * CH * P]
                    .rearrange("c p n -> p c n"))
                posc = ap.tile([P, CH, 2 * AFZ], F32)
                nc.scalar.dma_start(posc[:], posf[:, cc * CH:(cc + 1) * CH, :])
                sc4 = ap.tile([P, CH, 2 * AFZ], BF16)
                nc.scalar.activation(sc4[:], posc[:], AF.Sin)
                kvo = ap.tile([P, CH, 2 * HF], BF16)
                for j in range(CH):
                    kvps = aps.tile([P, 2 * HF], F32)
                    for c in range(2):
                        nc.tensor.matmul(kvps[:], x1c[:, c, j * P:(j + 1) * P],
                                         wkvb[:, c, :], start=(c == 0),
                                         stop=(c == 1))
                    # one K|V copy to bf16 (Act); RoPE then overwrites K half
                    nc.scalar.copy(kvo[:, j, :], kvps[:])
                    cosb = sc4[:, j, None, 0:AFZ].to_broadcast([P, AHZ, AFZ])
                    ssinb = sc4[:, j, None, AFZ:2 * AFZ].to_broadcast(
                        [P, AHZ, AFZ])
                    kb = kvo[:, j, 0:HF]
                    kh = kb.rearrange("p (h f) -> p h f", h=AHZ)
                    krot = kb.rearrange("p (h two g) -> p h two g", h=AHZ,
                                        two=2)
                    ss2 = ssinb.rearrange("p h (two g) -> p h two g", two=2)
                    t1 = ap.tile([P, AHZ, AFZ], BF16, tag="t1")
                    nc.vector.tensor_tensor(t1[:], kh, cosb, op=OP.mult)
                    t23 = ap.tile([P, AHZ, 2, AFZ // 2], BF16, tag="t23")
                    nc.vector.tensor_tensor(t23[:, :, 0, :], krot[:, :, 1, :],
                                            ss2[:, :, 0, :], op=OP.mult)
                    nc.vector.tensor_tensor(t23[:, :, 1, :], krot[:, :, 0, :],
                                            ss2[:, :, 1, :], op=OP.mult)
                    nc.vector.tensor_tensor(
                        kvo[:, j, 0:HF].rearrange("p (h f) -> p h f", h=AHZ),
                        t1[:], t23[:].rearrange("p h two g -> p h (two g)"),
                        op=OP.add)
                nc.sync.dma_start(
                    Tt[cc * CH * P:(cc + 1) * CH * P, :]
                    .rearrange("(j p) f -> p j f", p=P), kvo[:])

            # own-shard sincos (still Sin table)
            sc_own = const.tile([P, NT2, 2 * AFZ], BF16)
            for q in range(NT2 // CH):
                po = ap.tile([P, CH, 2 * AFZ], F32, tag="po")
                nc.scalar.dma_start(po[:], poso[:, q * CH:(q + 1) * CH, :])
                nc.scalar.activation(sc_own[:, q * CH:(q + 1) * CH, :], po[:],
                                     AF.Sin)

        # =============== phase C: attention over own shard ===============
        with tc.tile_pool(name="cpool", bufs=2) as cp, \
             tc.tile_pool(name="cdve", bufs=1) as cd, \
             tc.tile_pool(name="cgath", bufs=3) as cg, \
             tc.tile_pool(name="cx2", bufs=3) as cx, \
             tc.tile_pool(name="cpsum", bufs=2, space="PSUM") as cps, \
             tc.tile_pool(name="cpsum1", bufs=1, space="PSUM") as cps1:
            eis = const.tile([P, NT2, KZ], I32)
            nc.sync.dma_start(eis[:], eit[:, :, :])

            def load_x2tt(tt):
                x2l = cx.tile([P, 2, KZ, P], BF16, tag="x2tt")
                nc.sync.dma_start(x2l[:], x2t[:, :, tt, :, :])
                return x2l

            x2q = [load_x2tt(0), load_x2tt(1)]
            chunks = {}

            def stage1(t):
                t4 = t % CH
                if t4 == 0:
                    x1q = cp.tile([P, 2, CH * P], BF16, tag="x1q")
                    nc.sync.dma_start(x1q[:],
                                      x1qo[:, :, t * P:(t + CH) * P])
                    x1rc = cp.tile([P, CH, IFZ], F32, tag="x1rc")
                    nc.sync.dma_start(x1rc[:],
                                      x1rt[:, t:t + CH, :])
                    outw = cp.tile([P, CH, IFZ], F32, tag="outw")
                    chunks[t // CH] = (x1q, x1rc, outw)
                x1q, _, _ = chunks[t // CH]

                # ---- prefetch + gathers
                x2tt = x2q[t % 2]
                if t + 2 < NT2:
                    x2q[t % 2] = load_x2tt(t + 2)
                kvg = cg.tile([P, KZ, 2 * HF], BF16, tag="kvg")
                for j in range(KZ):
                    nc.gpsimd.indirect_dma_start(
                        out=kvg[:, j, :], out_offset=None, in_=Tt[:],
                        in_offset=IndirectOffsetOnAxis(
                            ap=eis[:, t, j:j + 1], axis=0))

                # ---- q/gate matmuls
                qg = cps.tile([P, 2 * HF], F32, tag="qg")
                for c in range(2):
                    nc.tensor.matmul(qg[:], x1q[:, c, t4 * P:(t4 + 1) * P],
                                     wqgb[:, c, :], start=(c == 0),
                                     stop=(c == 1))

                # ---- bias2 pre: coll[n, k, 0:8]=x2@(g*Wb), [n,k,8]=mean
                coll = cps.tile([P, KZ, 16], F32, tag="coll")
                for k in range(KZ):
                    for c in range(2):
                        nc.tensor.matmul(coll[:, k, :], x2tt[:, c, k, :],
                                         wbb[:, c, :], start=(c == 0),
                                         stop=(c == 1))

                # ---- sum of squares via ones-stationary matmul
                x2sq = cp.tile([P, 2, KZ, P], BF16, tag="x2sq")
                nc.scalar.activation(x2sq[:], x2tt[:], AF.Square)
                # chunk q (n-group) -> psum row {0,32}[q%2], bank half q//2
                ssT = cps1.tile([33, 2 * 2 * HF], F32, tag="ssT")
                x2v = x2sq[:].rearrange("p c k n -> p c n k")
                for q in range(4):
                    r, b = 32 * (q % 2), 2 * HF * (q // 2)
                    for c in range(2):
                        nc.tensor.matmul(
                            ssT[r:r + 1, b:b + 2 * HF],
                            ones1[:], x2v[:, c, 32 * q:32 * (q + 1), :],
                            start=(c == 0), stop=(c == 1))
                sst_sb = cp.tile([33, 2 * 2 * HF], F32, tag="sst_sb")
                nc.scalar.copy(sst_sb[0:1, :], ssT[0:1, :])
                nc.scalar.copy(sst_sb[32:33, :], ssT[32:33, :])
                ssq = cp.tile([P, KZ], F32, tag="ssq")
                for q in range(4):
                    r, b = 32 * (q % 2), 2 * HF * (q // 2)
                    nc.scalar.dma_start(
                        ssq[32 * q:32 * (q + 1), :]
                        .rearrange("p (o k) -> p o k", o=1),
                        sst_sb[r:r + 1, b:b + 2 * HF]
                        .rearrange("o (n k) -> o n k", n=32))

                # ---- RoPE(q) (reads qg PSUM f32)
                qh = cp.tile([P, AHZ, AFZ], BF16, tag="qh")
                cosb = sc_own[:, t, None, 0:AFZ].to_broadcast([P, AHZ, AFZ])
                ssinb = sc_own[:, t, None, AFZ:2 * AFZ].to_broadcast(
                    [P, AHZ, AFZ])
                qv = qg[:, 0:HF].rearrange("p (h f) -> p h f", h=AHZ)
                qrot = qg[:, 0:HF].rearrange("p (h two g) -> p h two g",
                                             h=AHZ, two=2)
                tq1 = cp.tile([P, AHZ, AFZ], BF16, tag="tq1")
                nc.vector.tensor_tensor(tq1[:], qv, cosb, op=OP.mult)
                tq2 = cp.tile([P, AHZ, 2, AFZ // 2], BF16, tag="tq2")
                ss2 = ssinb.rearrange("p h (two g) -> p h two g", two=2)
                nc.vector.tensor_tensor(tq2[:, :, 0, :], qrot[:, :, 1, :],
                                        ss2[:, :, 0, :], op=OP.mult)
                nc.vector.tensor_tensor(tq2[:, :, 1, :], qrot[:, :, 0, :],
                                        ss2[:, :, 1, :], op=OP.mult)
                nc.vector.tensor_tensor(
                    qh[:], tq1[:],
                    tq2[:].rearrange("p h two g -> p h (two g)"), op=OP.add)

                # ---- gate = sigmoid(x) = 0.5*tanh(0.5x)+0.5
                xg = cp.tile([P, HF], F32, tag="xg")
                nc.vector.tensor_tensor(xg[:], qg[:, HF:2 * HF], bg_r,
                                        op=OP.add)
                th = cp.tile([P, HF], BF16, tag="th")
                nc.scalar.activation(th[:], xg[:], AF.Tanh, scale=0.5)
                gate = cp.tile([P, HF], BF16, tag="gate")
                nc.vector.tensor_scalar(gate[:], th[:], 0.5, 0.5,
                                        op0=OP.mult, op1=OP.add)

                # ---- scores
                kview = kvg[:, :, 0:HF].rearrange("p k (h f) -> p k h f",
                                                  h=AHZ)
                qb = qh[:, None, :, :].to_broadcast([P, KZ, AHZ, AFZ])
                prod = cd.tile([P, KZ, AHZ, AFZ], BF16, tag="prod")
                nc.vector.tensor_tensor(prod[:], kview, qb, op=OP.mult)
                # f-sum via bf16 TT halving tree
                sA = cd.tile([P, KZ, AHZ, 16], BF16, tag="sA")
                nc.vector.tensor_tensor(sA[:], prod[:, :, :, 0:16],
                                        prod[:, :, :, 16:32], op=OP.add)
                sB = cd.tile([P, KZ, AHZ, 8], BF16, tag="sB")
                nc.vector.tensor_tensor(sB[:], sA[:, :, :, 0:8],
                                        sA[:, :, :, 8:16], op=OP.add)
                sC = cd.tile([P, KZ, AHZ, 4], BF16, tag="sC")
                nc.vector.tensor_tensor(sC[:], sB[:, :, :, 0:4],
                                        sB[:, :, :, 4:8], op=OP.add)
                sD = cd.tile([P, KZ, AHZ, 2], BF16, tag="sD")
                nc.vector.tensor_tensor(sD[:], sC[:, :, :, 0:2],
                                        sC[:, :, :, 2:4], op=OP.add)
                sco = cp.tile([P, KZ, AHZ], BF16, tag="sco")
                nc.vector.tensor_tensor(sco[:], sD[:, :, :, 0],
                                        sD[:, :, :, 1], op=OP.add)
                sco2 = cp.tile([P, KZ, AHZ], F32, tag="sco2")

                # ---- bias2 terms
                mu = cp.tile([P, KZ], F32, tag="mu")
                nc.vector.tensor_scalar_mul(mu[:], coll[:, :, 8], 1.0)
                msq = cp.tile([P, KZ], F32, tag="msq")
                nc.vector.tensor_tensor(msq[:], mu[:], mu[:], op=OP.mult)
                var = cp.tile([P, KZ], F32, tag="var")
                nc.vector.scalar_tensor_tensor(var[:], ssq[:], 1.0 / IFZ,
                                               msq[:], op0=OP.mult,
                                               op1=OP.subtract)
                nc.vector.tensor_scalar(var[:], var[:], EPS, None, op0=OP.add)
                rstd = _newton_rsqrt(nc, cp, var[:], KZ, "rsb")
                t1b = cp.tile([P, KZ, AHZ], F32, tag="t1b")
                nc.vector.tensor_tensor(
                    t1b[:], coll[:, :, 8:9].to_broadcast([P, KZ, AHZ]),
                    sg_r[:, None, :].to_broadcast([P, KZ, AHZ]), op=OP.mult)
                t2b = cp.tile([P, KZ, AHZ], F32, tag="t2b")
                nc.vector.tensor_tensor(t2b[:], coll[:, :, 0:AHZ], t1b[:],
                                        op=OP.subtract)
                nc.vector.tensor_tensor(
                    t2b[:], t2b[:],
                    rstd[:, :, None].to_broadcast([P, KZ, AHZ]), op=OP.mult)
                nc.vector.tensor_tensor(sco2[:], sco[:], t2b[:], op=OP.add)
                nc.vector.tensor_tensor(
                    sco2[:], sco2[:],
                    tb_r[:, None, :].to_broadcast([P, KZ, AHZ]), op=OP.add)

                # ---- softmax exp on a broadcast view (weighted-V gets 2x)
                ee = cp.tile([P, KZ, AHZ, AFZ], BF16, tag="ee")
                nc.scalar.activation(
                    ee[:],
                    sco2[:, :, :, None].to_broadcast([P, KZ, AHZ, AFZ]),
                    AF.Exp)
                return dict(t=t, kvg=kvg, ee=ee, gate=gate)

            def stage2(s):
                t = s["t"]
                t4 = t % CH
                kvg, ee, gate = s["kvg"], s["ee"], s["gate"]
                _, x1rc, outw = chunks[t // CH]

                rsum = cp.tile([P, AHZ], F32, tag="rsum")
                nc.vector.tensor_reduce(rsum[:],
                                        ee[:, :, :, 0].rearrange(
                                            "p k h -> p h k"),
                                        axis=mybir.AxisListType.X, op=OP.add)
                rinv = cp.tile([P, AHZ], F32, tag="rinv")
                nc.vector.reciprocal(rinv[:], rsum[:])

                # ---- weighted V
                vview = kvg[:, :, HF:2 * HF].rearrange(
                    "p k (h f) -> p k h f", h=AHZ)
                wvt = cd.tile([P, KZ, AHZ, AFZ], BF16, tag="wvt")
                nc.vector.tensor_tensor(wvt[:], vview, ee[:], op=OP.mult)
                a1 = cd.tile([P, 8, AHZ, AFZ], BF16, tag="a1")
                nc.vector.tensor_tensor(a1[:], wvt[:, 0:8], wvt[:, 8:16],
                                        op=OP.add)
                a2 = cd.tile([P, 4, AHZ, AFZ], BF16, tag="a2")
                nc.vector.tensor_tensor(a2[:], a1[:, 0:4], a1[:, 4:8],
                                        op=OP.add)
                a3 = cd.tile([P, 2, AHZ, AFZ], BF16, tag="a3")
                nc.vector.tensor_tensor(a3[:], a2[:, 0:2], a2[:, 2:4],
                                        op=OP.add)
                au = cp.tile([P, AHZ, AFZ], BF16, tag="au")
                nc.vector.tensor_tensor(au[:], a3[:, 0], a3[:, 1], op=OP.add)

                # ---- att = au * gate * rinv
                gsc = cp.tile([P, AHZ, AFZ], BF16, tag="gsc")
                nc.vector.tensor_tensor(
                    gsc[:], gate[:].rearrange("p (h f) -> p h f", h=AHZ),
                    rinv[:, :, None].to_broadcast([P, AHZ, AFZ]), op=OP.mult)
                att = cp.tile([P, HF], BF16, tag="att")
                nc.vector.tensor_tensor(
                    att[:].rearrange("p (h f) -> p h f", h=AHZ), au[:],
                    gsc[:], op=OP.mult)

                # ---- back matmul
                attT = cp.tile([P, 2, P], BF16, tag="attT")
                nc.sync.dma_start_transpose(attT[:], att[:])
                bout = cps.tile([P, IFZ], F32, tag="bout")
                for c in range(2):
                    nc.tensor.matmul(bout[:], attT[:, c, :], wbackb[:, c, :],
                                     start=(c == 0), stop=(c == 1))

                # ---- residual + final layernorm
                res = cp.tile([P, IFZ], F32, tag="res")
                nc.vector.scalar_tensor_tensor(res[:], x1rc[:, t4, :],
                                               math.sqrt(2.0), bout[:],
                                               op0=OP.mult, op1=OP.add)
                nc.vector.tensor_tensor(res[:], res[:], bback_r, op=OP.add)
                smean = cp.tile([P, 1], F32, tag="smean")
                nc.vector.tensor_reduce(smean[:], res[:],
                                        axis=mybir.AxisListType.X, op=OP.add)
                scr = cp.tile([P, IFZ], BF16, tag="scr")
                ssf = cp.tile([P, 1], F32, tag="ssf")
                nc.scalar.activation(scr[:], res[:], AF.Square,
                                     accum_out=ssf[:])
                meanf = cp.tile([P, 1], F32, tag="meanf")
                nc.vector.tensor_scalar_mul(meanf[:], smean[:], 1.0 / IFZ)
                msqf = cp.tile([P, 1], F32, tag="msqf")
                nc.vector.tensor_tensor(msqf[:], meanf[:], meanf[:],
                                        op=OP.mult)
                varf = cp.tile([P, 1], F32, tag="varf")
                nc.vector.scalar_tensor_tensor(varf[:], ssf[:], 1.0 / IFZ,
                                               msqf[:], op0=OP.mult,
                                               op1=OP.subtract)
                nc.vector.tensor_scalar(varf[:], varf[:], EPS, None,
                                        op0=OP.add)
                rstdf = _newton_rsqrt(nc, cp, varf[:], 1, "rsf")
                nbias = cp.tile([P, 1], F32, tag="nbias")
                nc.vector.scalar_tensor_tensor(nbias[:], meanf[:], -1.0,
                                               rstdf[:], op0=OP.mult,
                                               op1=OP.mult)
                xn = cp.tile([P, IFZ], F32, tag="xn")
                nc.scalar.activation(xn[:], res[:], AF.Identity,
                                     scale=rstdf[:], bias=nbias[:])
                nc.vector.tensor_tensor(outw[:, t4, :], xn[:], lng_r,
                                        op=OP.mult)
                nc.vector.tensor_tensor(outw[:, t4, :], outw[:, t4, :],
                                        lnb_r, op=OP.add)
                if t4 == CH - 1:
                    nc.sync.dma_start(
                        out[(t - t4) * P:(t + 1) * P, :]
                        .rearrange("(j p) f -> p j f", p=P), outw[:])

            prev = None
            for t in range(NT2):
                cur = stage1(t)
                if prev is not None:
                    stage2(prev)
                prev = cur
            stage2(prev)

    nc.compile()
    return nc


_NC_CACHE = {}


def _get_nc(n_pad=NP, n_shard=NSH, n_cores=N_CORES):
    key = (n_pad, n_shard, n_cores)
    if key not in _NC_CACHE:
        _NC_CACHE[key] = build_nc(n_cores)
    return _NC_CACHE[key]


def _red(x):
    return (x - 2 * math.pi * np.round(x / (2 * math.pi))).astype(np.float32)


def _pos64(pos):
    """[cos-args | signed-sin-args]: Sin of this gives [cos | ssin] where
    ssin[f<16] = -sin, ssin[f>=16] = +sin."""
    n = pos.shape[0]
    o = np.zeros((n, 2 * AFZ), np.float32)
    o[:, 0:AFZ] = _red(pos + math.pi / 2)
    o[:, AFZ:AFZ + 16] = _red(pos[:, 0:16] + math.pi)
    o[:, AFZ + 16:2 * AFZ] = _red(pos[:, 16:32])
    return o


def make_in_maps(x_1, x_2, pos_emb, edge_index, Wq, Wk, Wv, Wb, bln_g, bln_b,
                 Wg, bg, Wback, bback, ln1_g, ln1_b, n_cores=N_CORES):
    x_1 = np.asarray(x_1, np.float32)
    x_2 = np.asarray(x_2, np.float32)
    pos_emb = np.asarray(pos_emb, np.float32)
    edge_index = np.asarray(edge_index).astype(np.int32)
    n = x_1.shape[0]
    n_shard = n // n_cores     # 2500

    # global tensors (shared by all cores)
    x1p = np.zeros((NP, IFZ), np.float32)
    x1p[:n] = x_1
    x1t = np.ascontiguousarray(
        x1p.T.reshape(2, P, NP), dtype=BF)           # x1t[c, f, n]
    p64 = np.zeros((NP, 2 * AFZ), np.float32)
    p64[:n] = _pos64(pos_emb)
    posf = np.ascontiguousarray(
        p64.reshape(NT1, P, 2 * AFZ).transpose(1, 0, 2))  # [p, t, 64]

    s = 1.0 / math.sqrt(AFZ)
    wkv = np.concatenate([np.asarray(Wk), np.asarray(Wv)], axis=1) \
        .reshape(2, P, 2 * HF).astype(BF)
    wkv = np.ascontiguousarray(wkv.transpose(1, 0, 2))   # [f, c, 512]
    wqg = np.concatenate([np.asarray(Wq) * s, np.asarray(Wg)], axis=1) \
        .reshape(2, P, 2 * HF).astype(BF)
    wqg = np.ascontiguousarray(wqg.transpose(1, 0, 2))
    wb = np.zeros((IFZ, 16), np.float32)
    wb[:, 0:AHZ] = np.asarray(bln_g)[:, None] * np.asarray(Wb)
    wb[:, AHZ] = 1.0 / IFZ
    wb16 = np.ascontiguousarray(
        wb.reshape(2, P, 16).astype(BF).transpose(1, 0, 2))
    wbk = np.asarray(Wback).reshape(2, P, IFZ).astype(BF)
    wback = np.ascontiguousarray(wbk.transpose(1, 0, 2))
    vecs = np.zeros((1, 5 * IFZ + 16), np.float32)
    vecs[0, 0:IFZ] = np.asarray(bg)
    vecs[0, IFZ:2 * IFZ] = np.asarray(ln1_g)
    vecs[0, 2 * IFZ:3 * IFZ] = np.asarray(ln1_b)
    vecs[0, 3 * IFZ:4 * IFZ] = np.asarray(bback)
    vecs[0, 5 * IFZ:5 * IFZ + 8] = np.asarray(bln_g) @ np.asarray(Wb)
    vecs[0, 5 * IFZ + 8:5 * IFZ + 16] = np.asarray(bln_b) @ np.asarray(Wb)

    common = dict(x1t=x1t, posf=posf, wkv=wkv, wqg=wqg, wb16=wb16,
                  wback=wback, vecs=vecs)

    in_maps = []
    for c in range(n_cores):
        lo = c * n_shard
        # poso [p, t, 64]
        po = np.zeros((NSH, 2 * AFZ), np.float32)
        po[:n_shard] = _pos64(pos_emb[lo:lo + n_shard])
        poso = np.ascontiguousarray(
            po.reshape(NT2, P, 2 * AFZ).transpose(1, 0, 2))
        # x2t [f, c2, t, k, m]
        x2s = np.zeros((NSH, KZ, IFZ), np.float32)
        x2s[:n_shard] = x_2[lo:lo + n_shard]
        # [t, m, k, c2, f] -> [f, c2, t, k, m]
        x2r = x2s.reshape(NT2, P, KZ, 2, P).astype(BF)
        x2tc = np.ascontiguousarray(x2r.transpose(4, 3, 0, 2, 1))
        # eit [p, t, k]
        ei = np.zeros((NSH, KZ), np.int32)
        ei[:n_shard] = edge_index[lo:lo + n_shard]
        eit = np.ascontiguousarray(
            ei.reshape(NT2, P, KZ).transpose(1, 0, 2))
        # x1rt [p, t, 256]
        x1r = np.zeros((NSH, IFZ), np.float32)
        x1r[:n_shard] = x_1[lo:lo + n_shard]
        x1rt = np.ascontiguousarray(
            x1r.reshape(NT2, P, IFZ).transpose(1, 0, 2))
        # x1qo [f, c2, m] (own shard, feature-major)
        x1qo = np.ascontiguousarray(
            x1r.T.reshape(2, P, NSH).transpose(1, 0, 2).astype(BF))
        m = dict(common)
        m.update(poso=poso, x2t=x2tc, eit=eit, x1rt=x1rt, x1qo=x1qo)
        in_maps.append(m)
    return in_maps, NP, n_shard


def kernel(**inputs):
    x_1 = np.asarray(inputs["x_1"], np.float32)
    n = x_1.shape[0]
    n_cores = N_CORES
    n_shard = n // n_cores
    in_maps, _, _ = make_in_maps(**inputs)
    nc = _get_nc(NP, NSH, n_cores)
    res = run_bass_kernel_spmd(nc, in_maps, core_ids=list(range(n_cores)),
                               trace=False)
    out = np.concatenate(
        [res.results[c]["out"][:n_shard] for c in range(n_cores)], axis=0)
    return out[:n].astype(np.float32)


# revision 31
# speedup vs baseline: 1.0793x; 1.0793x over previous
"""Trainium2 Bass kernel for nn_NodeAttention (gnn_message_passing), v2.

Strategy (8 cores, data-parallel over nodes; weights + x_1 replicated):

Phase A (per core): build bf16 table T[n] = [RoPE(x_1@Wk) | x_1@Wv] for all
  20480 (padded) nodes. Host supplies x_1 pre-transposed (feature-major) so
  the stationary loads need no on-chip transpose; 4-tile-batched DMAs; RoPE
  as 3 DVE ops using a phase-shifted sin table (one Sin activation yields
  [cos | signed-sin]); V copied out of PSUM on Act/Pool.

Phase C (per core, 20 tiles of its padded 2560-node shard): per tile
  - 16 indirect row gathers (neighbor K|V rows, 1KB each) on the SWDGE
  - q/gate matmuls (stationary = host-transposed x_1 slice)
  - bias2 = LN(x_2)@Wb via algebraic refactor:
      rstd*(x2@(g*Wb) - mean*(g@Wb)) + b@Wb
    with mean as an extra matmul column (node-stationary, using host
    feature-major x_2) and sum-of-squares via a ones-stationary matmul on
    x_2^2 whose [1,2048] transposed result is reshaped node-major by a
    small SBUF->SBUF DMA.
  - rstd via bit-trick + Newton rsqrt on DVE (no Sqrt activation table)
  - sigmoid gate via tanh (same activation table as Exp)
  - scores/softmax/weighted-V elementwise on DVE, output matmul, final LN.

Activation tables: phase A uses only Sin/Copy, phase C only
Exp/Tanh/Square/Copy/Identity -> exactly two table loads.
"""
import sys, math
if "/opt/trn_rl_repo" not in sys.path:
    sys.path.insert(0, "/opt/trn_rl_repo")

import numpy as np
import ml_dtypes
from contextlib import ExitStack

import concourse.bass as bass
import concourse.tile as tile
from concourse import bacc, mybir
from concourse.bass import IndirectOffsetOnAxis
from concourse.bass_utils import run_bass_kernel_spmd

P = 128
KZ, IFZ, AHZ, AFZ = 16, 256, 8, 32
HF = AHZ * AFZ          # 256
EPS = 1e-5
F32 = mybir.dt.float32
BF16 = mybir.dt.bfloat16
I32 = mybir.dt.int32
AF = mybir.ActivationFunctionType
OP = mybir.AluOpType
N_CORES = 8
N_FULL = 20000
NP = 20480              # padded table rows (160 tiles)
NT1 = NP // P           # 160
CH = 4                  # phase-A tiles per DMA chunk
NCH = NT1 // CH         # 40
NSH = 2560              # padded shard rows (20 tiles)
NT2 = NSH // P          # 20

BF = ml_dtypes.bfloat16
MAGIC = 0x5F3759DF


def _newton_rsqrt(nc, pool, v_ap, n_free, tag):
    """rstd = 1/sqrt(v) on DVE via bit-trick seed + 2 Newton iterations.
    v_ap: [P, n_free] f32 AP (must be a plain SBUF tile view)."""
    ti = pool.tile([P, n_free], I32, tag=f"{tag}_i")
    nc.vector.tensor_scalar(ti[:], v_ap.bitcast(I32), 1, None,
                            op0=OP.logical_shift_right)
    nc.vector.tensor_scalar(ti[:], ti[:], -1, MAGIC, op0=OP.mult, op1=OP.add)
    y = pool.tile([P, n_free], F32, tag=f"{tag}_y")
    t2 = pool.tile([P, n_free], F32, tag=f"{tag}_t")
    yf = ti[:].bitcast(F32)
    nc.vector.tensor_tensor(t2[:], yf, yf, op=OP.mult)
    nc.vector.tensor_tensor(t2[:], t2[:], v_ap, op=OP.mult)
    nc.vector.tensor_scalar(t2[:], t2[:], -0.5, 1.5, op0=OP.mult, op1=OP.add)
    nc.vector.tensor_tensor(y[:], yf, t2[:], op=OP.mult)
    nc.vector.tensor_tensor(t2[:], y[:], y[:], op=OP.mult)
    nc.vector.tensor_tensor(t2[:], t2[:], v_ap, op=OP.mult)
    nc.vector.tensor_scalar(t2[:], t2[:], -0.5, 1.5, op0=OP.mult, op1=OP.add)
    nc.vector.tensor_tensor(y[:], y[:], t2[:], op=OP.mult)
    return y


def build_nc(n_cores=N_CORES):
    nc = bacc.Bacc("TRN2", target_bir_lowering=False, debug=False,
                   num_devices=n_cores)

    # ---------------- dram I/O ----------------
    x1t = nc.dram_tensor("x1t", [2, P, NP], BF16, kind="ExternalInput")
    x1qo = nc.dram_tensor("x1qo", [P, 2, NSH], BF16, kind="ExternalInput")
    posf = nc.dram_tensor("posf", [P, NT1, 2 * AFZ], F32, kind="ExternalInput")
    poso = nc.dram_tensor("poso", [P, NT2, 2 * AFZ], F32, kind="ExternalInput")
    x2t = nc.dram_tensor("x2t", [P, 2, NT2, KZ, P], BF16, kind="ExternalInput")
    eit = nc.dram_tensor("eit", [P, NT2, KZ], I32, kind="ExternalInput")
    x1rt = nc.dram_tensor("x1rt", [P, NT2, IFZ], F32, kind="ExternalInput")
    wkv = nc.dram_tensor("wkv", [P, 2, 2 * HF], BF16, kind="ExternalInput")
    wqg = nc.dram_tensor("wqg", [P, 2, 2 * HF], BF16, kind="ExternalInput")
    wb16 = nc.dram_tensor("wb16", [P, 2, 16], BF16, kind="ExternalInput")
    wback = nc.dram_tensor("wback", [P, 2, IFZ], BF16, kind="ExternalInput")
    vecs = nc.dram_tensor("vecs", [1, 5 * IFZ + 16], F32, kind="ExternalInput")
    # vecs layout: [bg(256) | lng(256) | lnb(256) | bback(256) | x?256 unused |
    #               sg8(8) tb8(8)]
    out = nc.dram_tensor("out", [NSH, IFZ], F32, kind="ExternalOutput")

    with tile.TileContext(nc) as tc, ExitStack() as ctx:
        const = ctx.enter_context(tc.tile_pool(name="const", bufs=1))
        dram = ctx.enter_context(tc.tile_pool(name="dram", bufs=1, space="DRAM"))

        # ---------------- constants ----------------
        wkvb = const.tile([P, 2, 2 * HF], BF16)
        nc.scalar.dma_start(wkvb[:], wkv[:, :, :])
        wqgb = const.tile([P, 2, 2 * HF], BF16)
        nc.scalar.dma_start(wqgb[:], wqg[:, :, :])
        wbb = const.tile([P, 2, 16], BF16)
        nc.scalar.dma_start(wbb[:], wb16[:, :, :])
        wbackb = const.tile([P, 2, IFZ], BF16)
        nc.scalar.dma_start(wbackb[:], wback[:, :, :])
        vec_r = const.tile([P, 5 * IFZ + 16], F32)
        nc.scalar.dma_start(vec_r[:], vecs[0:1, :].to_broadcast(
            [P, 5 * IFZ + 16]))
        bg_r = vec_r[:, 0:IFZ]
        lng_r = vec_r[:, IFZ:2 * IFZ]
        lnb_r = vec_r[:, 2 * IFZ:3 * IFZ]
        bback_r = vec_r[:, 3 * IFZ:4 * IFZ]
        sg_r = vec_r[:, 5 * IFZ:5 * IFZ + 8]
        tb_r = vec_r[:, 5 * IFZ + 8:5 * IFZ + 16]
        ones1 = const.tile([P, 1], BF16)
        nc.gpsimd.memset(ones1[:], 1.0)

        Tt = dram.tile([NP, 2 * HF], BF16)

        # =============== phase A: build K|V table ===============
        with tc.tile_pool(name="apool", bufs=3) as ap, \
             tc.tile_pool(name="apsum", bufs=3, space="PSUM") as aps:
            for cc in range(NCH):
                x1c = ap.tile([P, 2, CH * P], BF16)
                nc.sync.dma_start(
                    x1c[:], x1t[:, :, cc * CH * P:(cc + 1) * CH * P]
                    .rearrange("c p n -> p c n"))
                posc = ap.tile([P, CH, 2 * AFZ], F32)
                nc.scalar.dma_start(posc[:], posf[:, cc * CH:(cc + 1) * CH, :])
                sc4 = ap.tile([P, CH, 2 * AFZ], BF16)
                nc.scalar.activation(sc4[:], posc[:], AF.Sin)
                kvo = ap.tile([P, CH, 2 * HF], BF16)
                for j in range(CH):
                    kvps = aps.tile([P, 2 * HF], F32)
                    for c in range(2):
                        nc.tensor.matmul(kvps[:], x1c[:, c, j * P:(j + 1) * P],
                                         wkvb[:, c, :], start=(c == 0),
                                         stop=(c == 1))
                    # one K|V copy to bf16 (Act); RoPE then overwrites K half
                    nc.scalar.copy(kvo[:, j, :], kvps[:])
                    cosb = sc4[:, j, None, 0:AFZ].to_broadcast([P, AHZ, AFZ])
                    ssinb = sc4[:, j, None, AFZ:2 * AFZ].to_broadcast(
                        [P, AHZ, AFZ])
                    kb = kvo[:, j, 0:HF]
                    kh = kb.rearrange("p (h f) -> p h f", h=AHZ)
                    krot = kb.rearrange("p (h two g) -> p h two g", h=AHZ,
                                        two=2)
                    ss2 = ssinb.rearrange("p h (two g) -> p h two g", two=2)
                    t1 = ap.tile([P, AHZ, AFZ], BF16, tag="t1")
                    nc.vector.tensor_tensor(t1[:], kh, cosb, op=OP.mult)
                    t23 = ap.tile([P, AHZ, 2, AFZ // 2], BF16, tag="t23")
                    nc.vector.tensor_tensor(t23[:, :, 0, :], krot[:, :, 1, :],
                                            ss2[:, :, 0, :], op=OP.mult)
                    nc.vector.tensor_tensor(t23[:, :, 1, :], krot[:, :, 0, :],
                                            ss2[:, :, 1, :], op=OP.mult)
                    nc.vector.tensor_tensor(
                        kvo[:, j, 0:HF].rearrange("p (h f) -> p h f", h=AHZ),
                        t1[:], t23[:].rearrange("p h two g -> p h (two g)"),
                        op=OP.add)
                nc.sync.dma_start(
                    Tt[cc * CH * P:(cc + 1) * CH * P, :]
                    .rearrange("(j p) f -> p j f", p=P), kvo[:])

            # own-shard sincos (still Sin table)
            sc_own = const.tile([P, NT2, 2 * AFZ], BF16)
            for q in range(NT2 // CH):
                po = ap.tile([P, CH, 2 * AFZ], F32, tag="po")
                nc.scalar.dma_start(po[:], poso[:, q * CH:(q + 1) * CH, :])
                nc.scalar.activation(sc_own[:, q * CH:(q + 1) * CH, :], po[:],
                                     AF.Sin)

        # =============== phase C: attention over own shard ===============
        with tc.tile_pool(name="cpool", bufs=2) as cp, \
             tc.tile_pool(name="cdve", bufs=1) as cd, \
             tc.tile_pool(name="cgath", bufs=3) as cg, \
             tc.tile_pool(name="cx2", bufs=3) as cx, \
             tc.tile_pool(name="cpsum", bufs=2, space="PSUM") as cps, \
             tc.tile_pool(name="cpsum1", bufs=1, space="PSUM") as cps1:
            eis = const.tile([P, NT2, KZ], I32)
            nc.sync.dma_start(eis[:], eit[:, :, :])

            def load_x2tt(tt):
                # split into 4 DMAs so gather transfers interleave on the
                # shared DMA engines instead of queueing behind one 3.2us copy
                x2l = cx.tile([P, 2, KZ, P], BF16, tag="x2tt")
                for kq in range(4):
                    nc.sync.dma_start(x2l[:, :, 4 * kq:4 * (kq + 1), :],
                                      x2t[:, :, tt, 4 * kq:4 * (kq + 1), :])
                return x2l

            x2q = [load_x2tt(0), load_x2tt(1)]
            chunks = {}

            def stage1(t):
                t4 = t % CH
                if t4 == 0:
                    x1q = cp.tile([P, 2, CH * P], BF16, tag="x1q")
                    nc.sync.dma_start(x1q[:],
                                      x1qo[:, :, t * P:(t + CH) * P])
                    x1rc = cp.tile([P, CH, IFZ], F32, tag="x1rc")
                    nc.sync.dma_start(x1rc[:],
                                      x1rt[:, t:t + CH, :])
                    outw = cp.tile([P, CH, IFZ], F32, tag="outw")
                    chunks[t // CH] = (x1q, x1rc, outw)
                x1q, _, _ = chunks[t // CH]

                # ---- prefetch + gathers
                x2tt = x2q[t % 2]
                if t + 2 < NT2:
                    x2q[t % 2] = load_x2tt(t + 2)
                kvg = cg.tile([P, KZ, 2 * HF], BF16, tag="kvg")
                for j in range(KZ):
                    nc.gpsimd.indirect_dma_start(
                        out=kvg[:, j, :], out_offset=None, in_=Tt[:],
                        in_offset=IndirectOffsetOnAxis(
                            ap=eis[:, t, j:j + 1], axis=0))

                # ---- q/gate matmuls
                qg = cps.tile([P, 2 * HF], F32, tag="qg")
                for c in range(2):
                    nc.tensor.matmul(qg[:], x1q[:, c, t4 * P:(t4 + 1) * P],
                                     wqgb[:, c, :], start=(c == 0),
                                     stop=(c == 1))

                # ---- bias2 pre: coll[n, k, 0:8]=x2@(g*Wb), [n,k,8]=mean
                coll = cps.tile([P, KZ, 16], F32, tag="coll")
                for k in range(KZ):
                    for c in range(2):
                        nc.tensor.matmul(coll[:, k, :], x2tt[:, c, k, :],
                                         wbb[:, c, :], start=(c == 0),
                                         stop=(c == 1))

                # ---- sum of squares via ones-stationary matmul
                x2sq = cp.tile([P, 2, KZ, P], BF16, tag="x2sq")
                nc.scalar.activation(x2sq[:], x2tt[:], AF.Square)
                # chunk q (n-group) -> psum row {0,32}[q%2], bank half q//2
                ssT = cps1.tile([33, 2 * 2 * HF], F32, tag="ssT")
                x2v = x2sq[:].rearrange("p c k n -> p c n k")
                for q in range(4):
                    r, b = 32 * (q % 2), 2 * HF * (q // 2)
                    for c in range(2):
                        nc.tensor.matmul(
                            ssT[r:r + 1, b:b + 2 * HF],
                            ones1[:], x2v[:, c, 32 * q:32 * (q + 1), :],
                            start=(c == 0), stop=(c == 1))
                sst_sb = cp.tile([33, 2 * 2 * HF], F32, tag="sst_sb")
                nc.scalar.copy(sst_sb[0:1, :], ssT[0:1, :])
                nc.scalar.copy(sst_sb[32:33, :], ssT[32:33, :])
                ssq = cp.tile([P, KZ], F32, tag="ssq")
                for q in range(4):
                    r, b = 32 * (q % 2), 2 * HF * (q // 2)
                    nc.scalar.dma_start(
                        ssq[32 * q:32 * (q + 1), :]
                        .rearrange("p (o k) -> p o k", o=1),
                        sst_sb[r:r + 1, b:b + 2 * HF]
                        .rearrange("o (n k) -> o n k", n=32))

                # ---- RoPE(q) (reads qg PSUM f32)
                qh = cp.tile([P, AHZ, AFZ], BF16, tag="qh")
                cosb = sc_own[:, t, None, 0:AFZ].to_broadcast([P, AHZ, AFZ])
                ssinb = sc_own[:, t, None, AFZ:2 * AFZ].to_broadcast(
                    [P, AHZ, AFZ])
                qv = qg[:, 0:HF].rearrange("p (h f) -> p h f", h=AHZ)
                qrot = qg[:, 0:HF].rearrange("p (h two g) -> p h two g",
                                             h=AHZ, two=2)
                tq1 = cp.tile([P, AHZ, AFZ], BF16, tag="tq1")
                nc.vector.tensor_tensor(tq1[:], qv, cosb, op=OP.mult)
                tq2 = cp.tile([P, AHZ, 2, AFZ // 2], BF16, tag="tq2")
                ss2 = ssinb.rearrange("p h (two g) -> p h two g", two=2)
                nc.vector.tensor_tensor(tq2[:, :, 0, :], qrot[:, :, 1, :],
                                        ss2[:, :, 0, :], op=OP.mult)
                nc.vector.tensor_tensor(tq2[:, :, 1, :], qrot[:, :, 0, :],
                                        ss2[:, :, 1, :], op=OP.mult)
                nc.vector.tensor_tensor(
                    qh[:], tq1[:],
                    tq2[:].rearrange("p h two g -> p h (two g)"), op=OP.add)

                # ---- gate = sigmoid(x) = 0.5*tanh(0.5x)+0.5
                xg = cp.tile([P, HF], F32, tag="xg")
                nc.vector.tensor_tensor(xg[:], qg[:, HF:2 * HF], bg_r,
                                        op=OP.add)
                th = cp.tile([P, HF], BF16, tag="th")
                nc.scalar.activation(th[:], xg[:], AF.Tanh, scale=0.5)
                gate = cp.tile([P, HF], BF16, tag="gate")
                nc.vector.tensor_scalar(gate[:], th[:], 0.5, 0.5,
                                        op0=OP.mult, op1=OP.add)

                # ---- scores
                kview = kvg[:, :, 0:HF].rearrange("p k (h f) -> p k h f",
                                                  h=AHZ)
                qb = qh[:, None, :, :].to_broadcast([P, KZ, AHZ, AFZ])
                prod = cd.tile([P, KZ, AHZ, AFZ], BF16, tag="prod")
                nc.vector.tensor_tensor(prod[:], kview, qb, op=OP.mult)
                # f-sum via bf16 TT halving tree
                sA = cd.tile([P, KZ, AHZ, 16], BF16, tag="sA")
                nc.vector.tensor_tensor(sA[:], prod[:, :, :, 0:16],
                                        prod[:, :, :, 16:32], op=OP.add)
                sB = cd.tile([P, KZ, AHZ, 8], BF16, tag="sB")
                nc.vector.tensor_tensor(sB[:], sA[:, :, :, 0:8],
                                        sA[:, :, :, 8:16], op=OP.add)
                sC = cd.tile([P, KZ, AHZ, 4], BF16, tag="sC")
                nc.vector.tensor_tensor(sC[:], sB[:, :, :, 0:4],
                                        sB[:, :, :, 4:8], op=OP.add)
                sD = cd.tile([P, KZ, AHZ, 2], BF16, tag="sD")
                nc.vector.tensor_tensor(sD[:], sC[:, :, :, 0:2],
                                        sC[:, :, :, 2:4], op=OP.add)
                sco = cp.tile([P, KZ, AHZ], BF16, tag="sco")
                nc.vector.tensor_tensor(sco[:], sD[:, :, :, 0],
                                        sD[:, :, :, 1], op=OP.add)
                sco2 = cp.tile([P, KZ, AHZ], F32, tag="sco2")

                # ---- bias2 terms
                mu = cp.tile([P, KZ], F32, tag="mu")
                nc.vector.tensor_scalar_mul(mu[:], coll[:, :, 8], 1.0)
                msq = cp.tile([P, KZ], F32, tag="msq")
                nc.vector.tensor_tensor(msq[:], mu[:], mu[:], op=OP.mult)
                var = cp.tile([P, KZ], F32, tag="var")
                nc.vector.scalar_tensor_tensor(var[:], ssq[:], 1.0 / IFZ,
                                               msq[:], op0=OP.mult,
                                               op1=OP.subtract)
                nc.vector.tensor_scalar(var[:], var[:], EPS, None, op0=OP.add)
                rstd = _newton_rsqrt(nc, cp, var[:], KZ, "rsb")
                t1b = cp.tile([P, KZ, AHZ], F32, tag="t1b")
                nc.vector.tensor_tensor(
                    t1b[:], coll[:, :, 8:9].to_broadcast([P, KZ, AHZ]),
                    sg_r[:, None, :].to_broadcast([P, KZ, AHZ]), op=OP.mult)
                t2b = cp.tile([P, KZ, AHZ], F32, tag="t2b")
                nc.vector.tensor_tensor(t2b[:], coll[:, :, 0:AHZ], t1b[:],
                                        op=OP.subtract)
                nc.vector.tensor_tensor(
                    t2b[:], t2b[:],
                    rstd[:, :, None].to_broadcast([P, KZ, AHZ]), op=OP.mult)
                nc.vector.tensor_tensor(sco2[:], sco[:], t2b[:], op=OP.add)
                nc.vector.tensor_tensor(
                    sco2[:], sco2[:],
                    tb_r[:, None, :].to_broadcast([P, KZ, AHZ]), op=OP.add)

                # ---- softmax exp on a broadcast view (weighted-V gets 2x)
                ee = cp.tile([P, KZ, AHZ, AFZ], BF16, tag="ee")
                nc.scalar.activation(
                    ee[:],
                    sco2[:, :, :, None].to_broadcast([P, KZ, AHZ, AFZ]),
                    AF.Exp)
                return dict(t=t, kvg=kvg, ee=ee, gate=gate)

            def stage2(s):
                t = s["t"]
                t4 = t % CH
                kvg, ee, gate = s["kvg"], s["ee"], s["gate"]
                _, x1rc, outw = chunks[t // CH]

                rsum = cp.tile([P, AHZ], F32, tag="rsum")
                nc.vector.tensor_reduce(rsum[:],
                                        ee[:, :, :, 0].rearrange(
                                            "p k h -> p h k"),
                                        axis=mybir.AxisListType.X, op=OP.add)
                rinv = cp.tile([P, AHZ], F32, tag="rinv")
                nc.vector.reciprocal(rinv[:], rsum[:])

                # ---- weighted V
                vview = kvg[:, :, HF:2 * HF].rearrange(
                    "p k (h f) -> p k h f", h=AHZ)
                wvt = cd.tile([P, KZ, AHZ, AFZ], BF16, tag="wvt")
                nc.vector.tensor_tensor(wvt[:], vview, ee[:], op=OP.mult)
                a1 = cd.tile([P, 8, AHZ, AFZ], BF16, tag="a1")
                nc.vector.tensor_tensor(a1[:], wvt[:, 0:8], wvt[:, 8:16],
                                        op=OP.add)
                a2 = cd.tile([P, 4, AHZ, AFZ], BF16, tag="a2")
                nc.vector.tensor_tensor(a2[:], a1[:, 0:4], a1[:, 4:8],
                                        op=OP.add)
                a3 = cd.tile([P, 2, AHZ, AFZ], BF16, tag="a3")
                nc.vector.tensor_tensor(a3[:], a2[:, 0:2], a2[:, 2:4],
                                        op=OP.add)
                au = cp.tile([P, AHZ, AFZ], BF16, tag="au")
                nc.vector.tensor_tensor(au[:], a3[:, 0], a3[:, 1], op=OP.add)

                # ---- att = au * gate * rinv
                gsc = cp.tile([P, AHZ, AFZ], BF16, tag="gsc")
                nc.vector.tensor_tensor(
                    gsc[:], gate[:].rearrange("p (h f) -> p h f", h=AHZ),
                    rinv[:, :, None].to_broadcast([P, AHZ, AFZ]), op=OP.mult)
                att = cp.tile([P, HF], BF16, tag="att")
                nc.vector.tensor_tensor(
                    att[:].rearrange("p (h f) -> p h f", h=AHZ), au[:],
                    gsc[:], op=OP.mult)

                # ---- back matmul
                attT = cp.tile([P, 2, P], BF16, tag="attT")
                nc.sync.dma_start_transpose(attT[:], att[:])
                bout = cps.tile([P, IFZ], F32, tag="bout")
                for c in range(2):
                    nc.tensor.matmul(bout[:], attT[:, c, :], wbackb[:, c, :],
                                     start=(c == 0), stop=(c == 1))

                # ---- residual + final layernorm
                res = cp.tile([P, IFZ], F32, tag="res")
                nc.vector.scalar_tensor_tensor(res[:], x1rc[:, t4, :],
                                               math.sqrt(2.0), bout[:],
                                               op0=OP.mult, op1=OP.add)
                nc.vector.tensor_tensor(res[:], res[:], bback_r, op=OP.add)
                smean = cp.tile([P, 1], F32, tag="smean")
                nc.vector.tensor_reduce(smean[:], res[:],
                                        axis=mybir.AxisListType.X, op=OP.add)
                scr = cp.tile([P, IFZ], BF16, tag="scr")
                ssf = cp.tile([P, 1], F32, tag="ssf")
                nc.scalar.activation(scr[:], res[:], AF.Square,
                                     accum_out=ssf[:])
                meanf = cp.tile([P, 1], F32, tag="meanf")
                nc.vector.tensor_scalar_mul(meanf[:], smean[:], 1.0 / IFZ)
                msqf = cp.tile([P, 1], F32, tag="msqf")
                nc.vector.tensor_tensor(msqf[:], meanf[:], meanf[:],
                                        op=OP.mult)
                varf = cp.tile([P, 1], F32, tag="varf")
                nc.vector.scalar_tensor_tensor(varf[:], ssf[:], 1.0 / IFZ,
                                               msqf[:], op0=OP.mult,
                                               op1=OP.subtract)
                nc.vector.tensor_scalar(varf[:], varf[:], EPS, None,
                                        op0=OP.add)
                rstdf = _newton_rsqrt(nc, cp, varf[:], 1, "rsf")
                nbias = cp.tile([P, 1], F32, tag="nbias")
                nc.vector.scalar_tensor_tensor(nbias[:], meanf[:], -1.0,
                                               rstdf[:], op0=OP.mult,
                                               op1=OP.mult)
                xn = cp.tile([P, IFZ], F32, tag="xn")
                nc.scalar.activation(xn[:], res[:], AF.Identity,
                                     scale=rstdf[:], bias=nbias[:])
                nc.vector.tensor_tensor(outw[:, t4, :], xn[:], lng_r,
                                        op=OP.mult)
                nc.vector.tensor_tensor(outw[:, t4, :], outw[:, t4, :],
                                        lnb_r, op=OP.add)
                if t4 == CH - 1:
                    nc.sync.dma_start(
                        out[(t - t4) * P:(t + 1) * P, :]
                        .rearrange("(j p) f -> p j f", p=P), outw[:])

            prev = None
            for t in range(NT2):
                cur = stage1(t)
                if prev is not None:
                    stage2(prev)
                prev = cur
            stage2(prev)

    nc.compile()
    return nc


_NC_CACHE = {}


def _get_nc(n_pad=NP, n_shard=NSH, n_cores=N_CORES):
    key = (n_pad, n_shard, n_cores)
    if key not in _NC_CACHE:
        _NC_CACHE[key] = build_nc(n_cores)
    return _NC_CACHE[key]


def _red(x):
    return (x - 2 * math.pi * np.round(x / (2 * math.pi))).astype(np.float32)


def _pos64(pos):
    """[cos-args | signed-sin-args]: Sin of this gives [cos | ssin] where
    ssin[f<16] = -sin, ssin[f>=16] = +sin."""
    n = pos.shape[0]
    o = np.zeros((n, 2 * AFZ), np.float32)
    o[:, 0:AFZ] = _red(pos + math.pi / 2)
    o[:, AFZ:AFZ + 16] = _red(pos[:, 0:16] + math.pi)
    o[:, AFZ + 16:2 * AFZ] = _red(pos[:, 16:32])
    return o


def make_in_maps(x_1, x_2, pos_emb, edge_index, Wq, Wk, Wv, Wb, bln_g, bln_b,
                 Wg, bg, Wback, bback, ln1_g, ln1_b, n_cores=N_CORES):
    x_1 = np.asarray(x_1, np.float32)
    x_2 = np.asarray(x_2, np.float32)
    pos_emb = np.asarray(pos_emb, np.float32)
    edge_index = np.asarray(edge_index).astype(np.int32)
    n = x_1.shape[0]
    n_shard = n // n_cores     # 2500

    # global tensors (shared by all cores)
    x1p = np.zeros((NP, IFZ), np.float32)
    x1p[:n] = x_1
    x1t = np.ascontiguousarray(
        x1p.T.reshape(2, P, NP), dtype=BF)           # x1t[c, f, n]
    p64 = np.zeros((NP, 2 * AFZ), np.float32)
    p64[:n] = _pos64(pos_emb)
    posf = np.ascontiguousarray(
        p64.reshape(NT1, P, 2 * AFZ).transpose(1, 0, 2))  # [p, t, 64]

    s = 1.0 / math.sqrt(AFZ)
    wkv = np.concatenate([np.asarray(Wk), np.asarray(Wv)], axis=1) \
        .reshape(2, P, 2 * HF).astype(BF)
    wkv = np.ascontiguousarray(wkv.transpose(1, 0, 2))   # [f, c, 512]
    wqg = np.concatenate([np.asarray(Wq) * s, np.asarray(Wg)], axis=1) \
        .reshape(2, P, 2 * HF).astype(BF)
    wqg = np.ascontiguousarray(wqg.transpose(1, 0, 2))
    wb = np.zeros((IFZ, 16), np.float32)
    wb[:, 0:AHZ] = np.asarray(bln_g)[:, None] * np.asarray(Wb)
    wb[:, AHZ] = 1.0 / IFZ
    wb16 = np.ascontiguousarray(
        wb.reshape(2, P, 16).astype(BF).transpose(1, 0, 2))
    wbk = np.asarray(Wback).reshape(2, P, IFZ).astype(BF)
    wback = np.ascontiguousarray(wbk.transpose(1, 0, 2))
    vecs = np.zeros((1, 5 * IFZ + 16), np.float32)
    vecs[0, 0:IFZ] = np.asarray(bg)
    vecs[0, IFZ:2 * IFZ] = np.asarray(ln1_g)
    vecs[0, 2 * IFZ:3 * IFZ] = np.asarray(ln1_b)
    vecs[0, 3 * IFZ:4 * IFZ] = np.asarray(bback)
    vecs[0, 5 * IFZ:5 * IFZ + 8] = np.asarray(bln_g) @ np.asarray(Wb)
    vecs[0, 5 * IFZ + 8:5 * IFZ + 16] = np.asarray(bln_b) @ np.asarray(Wb)

    common = dict(x1t=x1t, posf=posf, wkv=wkv, wqg=wqg, wb16=wb16,
                  wback=wback, vecs=vecs)

    in_maps = []
    for c in range(n_cores):
        lo = c * n_shard
        # poso [p, t, 64]
        po = np.zeros((NSH, 2 * AFZ), np.float32)
        po[:n_shard] = _pos64(pos_emb[lo:lo + n_shard])
        poso = np.ascontiguousarray(
            po.reshape(NT2, P, 2 * AFZ).transpose(1, 0, 2))
        # x2t [f, c2, t, k, m]
        x2s = np.zeros((NSH, KZ, IFZ), np.float32)
        x2s[:n_shard] = x_2[lo:lo + n_shard]
        # [t, m, k, c2, f] -> [f, c2, t, k, m]
        x2r = x2s.reshape(NT2, P, KZ, 2, P).astype(BF)
        x2tc = np.ascontiguousarray(x2r.transpose(4, 3, 0, 2, 1))
        # eit [p, t, k]
        ei = np.zeros((NSH, KZ), np.int32)
        ei[:n_shard] = edge_index[lo:lo + n_shard]
        eit = np.ascontiguousarray(
            ei.reshape(NT2, P, KZ).transpose(1, 0, 2))
        # x1rt [p, t, 256]
        x1r = np.zeros((NSH, IFZ), np.float32)
        x1r[:n_shard] = x_1[lo:lo + n_shard]
        x1rt = np.ascontiguousarray(
            x1r.reshape(NT2, P, IFZ).transpose(1, 0, 2))
        # x1qo [f, c2, m] (own shard, feature-major)
        x1qo = np.ascontiguousarray(
            x1r.T.reshape(2, P, NSH).transpose(1, 0, 2).astype(BF))
        m = dict(common)
        m.update(poso=poso, x2t=x2tc, eit=eit, x1rt=x1rt, x1qo=x1qo)
        in_maps.append(m)
    return in_maps, NP, n_shard


def kernel(**inputs):
    x_1 = np.asarray(inputs["x_1"], np.float32)
    n = x_1.shape[0]
    n_cores = N_CORES
    n_shard = n // n_cores
    in_maps, _, _ = make_in_maps(**inputs)
    nc = _get_nc(NP, NSH, n_cores)
    res = run_bass_kernel_spmd(nc, in_maps, core_ids=list(range(n_cores)),
                               trace=False)
    out = np.concatenate(
        [res.results[c]["out"][:n_shard] for c in range(n_cores)], axis=0)
    return out[:n].astype(np.float32)


# revision 35
# speedup vs baseline: 1.1576x; 1.0726x over previous
"""Trainium2 Bass kernel for nn_NodeAttention (gnn_message_passing), v2.

Strategy (8 cores, data-parallel over nodes; weights + x_1 replicated):

Phase A (per core): build bf16 table T[n] = [RoPE(x_1@Wk) | x_1@Wv] for all
  20480 (padded) nodes. Host supplies x_1 pre-transposed (feature-major) so
  the stationary loads need no on-chip transpose; 4-tile-batched DMAs; RoPE
  as 3 DVE ops using a phase-shifted sin table (one Sin activation yields
  [cos | signed-sin]); V copied out of PSUM on Act/Pool.

Phase C (per core, 20 tiles of its padded 2560-node shard): per tile
  - 16 indirect row gathers (neighbor K|V rows, 1KB each) on the SWDGE
  - q/gate matmuls (stationary = host-transposed x_1 slice)
  - bias2 = LN(x_2)@Wb via algebraic refactor:
      rstd*(x2@(g*Wb) - mean*(g@Wb)) + b@Wb
    with mean as an extra matmul column (node-stationary, using host
    feature-major x_2) and sum-of-squares via a ones-stationary matmul on
    x_2^2 whose [1,2048] transposed result is reshaped node-major by a
    small SBUF->SBUF DMA.
  - rstd via bit-trick + Newton rsqrt on DVE (no Sqrt activation table)
  - sigmoid gate via tanh (same activation table as Exp)
  - scores/softmax/weighted-V elementwise on DVE, output matmul, final LN.

Activation tables: phase A uses only Sin/Copy, phase C only
Exp/Tanh/Square/Copy/Identity -> exactly two table loads.
"""
import sys, math
if "/opt/trn_rl_repo" not in sys.path:
    sys.path.insert(0, "/opt/trn_rl_repo")

import numpy as np
import ml_dtypes
from contextlib import ExitStack

import concourse.bass as bass
import concourse.tile as tile
from concourse import bacc, mybir
from concourse.bass import IndirectOffsetOnAxis
from concourse.bass_utils import run_bass_kernel_spmd

P = 128
KZ, IFZ, AHZ, AFZ = 16, 256, 8, 32
HF = AHZ * AFZ          # 256
EPS = 1e-5
F32 = mybir.dt.float32
BF16 = mybir.dt.bfloat16
I32 = mybir.dt.int32
AF = mybir.ActivationFunctionType
OP = mybir.AluOpType
N_CORES = 8
N_FULL = 20000
NP = 20480              # padded table rows (160 tiles)
NT1 = NP // P           # 160
CH = 4                  # phase-A tiles per DMA chunk
NCH = NT1 // CH         # 40
NSH = 2560              # padded shard rows (20 tiles)
NT2 = NSH // P          # 20

BF = ml_dtypes.bfloat16
MAGIC = 0x5F3759DF


def _newton_rsqrt(nc, pool, v_ap, n_free, tag):
    """rstd = 1/sqrt(v) on DVE via bit-trick seed + 2 Newton iterations.
    v_ap: [P, n_free] f32 AP (must be a plain SBUF tile view)."""
    ti = pool.tile([P, n_free], I32, tag=f"{tag}_i")
    nc.vector.tensor_scalar(ti[:], v_ap.bitcast(I32), 1, None,
                            op0=OP.logical_shift_right)
    nc.vector.tensor_scalar(ti[:], ti[:], -1, MAGIC, op0=OP.mult, op1=OP.add)
    y = pool.tile([P, n_free], F32, tag=f"{tag}_y")
    t2 = pool.tile([P, n_free], F32, tag=f"{tag}_t")
    yf = ti[:].bitcast(F32)
    nc.vector.tensor_tensor(t2[:], yf, yf, op=OP.mult)
    nc.vector.tensor_tensor(t2[:], t2[:], v_ap, op=OP.mult)
    nc.vector.tensor_scalar(t2[:], t2[:], -0.5, 1.5, op0=OP.mult, op1=OP.add)
    nc.vector.tensor_tensor(y[:], yf, t2[:], op=OP.mult)
    nc.vector.tensor_tensor(t2[:], y[:], y[:], op=OP.mult)
    nc.vector.tensor_tensor(t2[:], t2[:], v_ap, op=OP.mult)
    nc.vector.tensor_scalar(t2[:], t2[:], -0.5, 1.5, op0=OP.mult, op1=OP.add)
    nc.vector.tensor_tensor(y[:], y[:], t2[:], op=OP.mult)
    return y


def build_nc(n_cores=N_CORES):
    nc = bacc.Bacc("TRN2", target_bir_lowering=False, debug=False,
                   num_devices=n_cores)

    # ---------------- dram I/O ----------------
    x1t = nc.dram_tensor("x1t", [2, P, NP], BF16, kind="ExternalInput")
    x1qo = nc.dram_tensor("x1qo", [P, 2, NSH], BF16, kind="ExternalInput")
    posf = nc.dram_tensor("posf", [P, NT1, 2 * AFZ], F32, kind="ExternalInput")
    poso = nc.dram_tensor("poso", [P, NT2, 2 * AFZ], F32, kind="ExternalInput")
    x2t = nc.dram_tensor("x2t", [P, 2, NT2, KZ, P], BF16, kind="ExternalInput")
    eit = nc.dram_tensor("eit", [P, NT2, KZ], I32, kind="ExternalInput")
    x1rt = nc.dram_tensor("x1rt", [P, NT2, IFZ], F32, kind="ExternalInput")
    wkv = nc.dram_tensor("wkv", [P, 2, 2 * HF], BF16, kind="ExternalInput")
    wqg = nc.dram_tensor("wqg", [P, 2, 2 * HF], BF16, kind="ExternalInput")
    wb16 = nc.dram_tensor("wb16", [P, 2, 16], BF16, kind="ExternalInput")
    wback = nc.dram_tensor("wback", [P, 2, IFZ], BF16, kind="ExternalInput")
    vecs = nc.dram_tensor("vecs", [1, 5 * IFZ + 16], F32, kind="ExternalInput")
    # vecs layout: [bg(256) | lng(256) | lnb(256) | bback(256) | x?256 unused |
    #               sg8(8) tb8(8)]
    out = nc.dram_tensor("out", [NSH, IFZ], F32, kind="ExternalOutput")

    with tile.TileContext(nc) as tc, ExitStack() as ctx:
        const = ctx.enter_context(tc.tile_pool(name="const", bufs=1))
        dram = ctx.enter_context(tc.tile_pool(name="dram", bufs=1, space="DRAM"))

        # ---------------- constants ----------------
        wkvb = const.tile([P, 2, 2 * HF], BF16)
        nc.scalar.dma_start(wkvb[:], wkv[:, :, :])
        wqgb = const.tile([P, 2, 2 * HF], BF16)
        nc.scalar.dma_start(wqgb[:], wqg[:, :, :])
        wbb = const.tile([P, 2, 16], BF16)
        nc.scalar.dma_start(wbb[:], wb16[:, :, :])
        wbackb = const.tile([P, 2, IFZ], BF16)
        nc.scalar.dma_start(wbackb[:], wback[:, :, :])
        vec_r = const.tile([P, 5 * IFZ + 16], F32)
        nc.scalar.dma_start(vec_r[:], vecs[0:1, :].to_broadcast(
            [P, 5 * IFZ + 16]))
        bg_r = vec_r[:, 0:IFZ]
        lng_r = vec_r[:, IFZ:2 * IFZ]
        lnb_r = vec_r[:, 2 * IFZ:3 * IFZ]
        bback_r = vec_r[:, 3 * IFZ:4 * IFZ]
        sg_r = vec_r[:, 5 * IFZ:5 * IFZ + 8]
        tb_r = vec_r[:, 5 * IFZ + 8:5 * IFZ + 16]
        ones1 = const.tile([P, 1], BF16)
        nc.gpsimd.memset(ones1[:], 1.0)

        Tt = dram.tile([NP, 2 * HF], BF16)

        # =============== phase A: build K|V table ===============
        with tc.tile_pool(name="apool", bufs=3) as ap, \
             tc.tile_pool(name="apsum", bufs=3, space="PSUM") as aps:
            for cc in range(NCH):
                x1c = ap.tile([P, 2, CH * P], BF16)
                nc.sync.dma_start(
                    x1c[:], x1t[:, :, cc * CH * P:(cc + 1) * CH * P]
                    .rearrange("c p n -> p c n"))
                posc = ap.tile([P, CH, 2 * AFZ], F32)
                nc.scalar.dma_start(posc[:], posf[:, cc * CH:(cc + 1) * CH, :])
                sc4 = ap.tile([P, CH, 2 * AFZ], BF16)
                nc.scalar.activation(sc4[:], posc[:], AF.Sin)
                kvo = ap.tile([P, CH, 2 * HF], BF16)
                for j in range(CH):
                    kvps = aps.tile([P, 2 * HF], F32)
                    for c in range(2):
                        nc.tensor.matmul(kvps[:], x1c[:, c, j * P:(j + 1) * P],
                                         wkvb[:, c, :], start=(c == 0),
                                         stop=(c == 1))
                    # one K|V copy to bf16 (Act); RoPE then overwrites K half
                    nc.scalar.copy(kvo[:, j, :], kvps[:])
                    cosb = sc4[:, j, None, 0:AFZ].to_broadcast([P, AHZ, AFZ])
                    ssinb = sc4[:, j, None, AFZ:2 * AFZ].to_broadcast(
                        [P, AHZ, AFZ])
                    kb = kvo[:, j, 0:HF]
                    kh = kb.rearrange("p (h f) -> p h f", h=AHZ)
                    krot = kb.rearrange("p (h two g) -> p h two g", h=AHZ,
                                        two=2)
                    ss2 = ssinb.rearrange("p h (two g) -> p h two g", two=2)
                    t1 = ap.tile([P, AHZ, AFZ], BF16, tag="t1")
                    nc.vector.tensor_tensor(t1[:], kh, cosb, op=OP.mult)
                    t23 = ap.tile([P, AHZ, 2, AFZ // 2], BF16, tag="t23")
                    nc.vector.tensor_tensor(t23[:, :, 0, :], krot[:, :, 1, :],
                                            ss2[:, :, 0, :], op=OP.mult)
                    nc.vector.tensor_tensor(t23[:, :, 1, :], krot[:, :, 0, :],
                                            ss2[:, :, 1, :], op=OP.mult)
                    nc.vector.tensor_tensor(
                        kvo[:, j, 0:HF].rearrange("p (h f) -> p h f", h=AHZ),
                        t1[:], t23[:].rearrange("p h two g -> p h (two g)"),
                        op=OP.add)
                nc.sync.dma_start(
                    Tt[cc * CH * P:(cc + 1) * CH * P, :]
                    .rearrange("(j p) f -> p j f", p=P), kvo[:])

            # own-shard sincos (still Sin table)
            sc_own = const.tile([P, NT2, 2 * AFZ], BF16)
            for q in range(NT2 // CH):
                po = ap.tile([P, CH, 2 * AFZ], F32, tag="po")
                nc.scalar.dma_start(po[:], poso[:, q * CH:(q + 1) * CH, :])
                nc.scalar.activation(sc_own[:, q * CH:(q + 1) * CH, :], po[:],
                                     AF.Sin)

        # =============== phase C: attention over own shard ===============
        with tc.tile_pool(name="cpool", bufs=2) as cp, \
             tc.tile_pool(name="cdve", bufs=1) as cd, \
             tc.tile_pool(name="cgath", bufs=2) as cg, \
             tc.tile_pool(name="cx2", bufs=3) as cx, \
             tc.tile_pool(name="cxs", bufs=1) as cx1, \
             tc.tile_pool(name="cpsum", bufs=2, space="PSUM") as cps, \
             tc.tile_pool(name="cpsum1", bufs=1, space="PSUM") as cps1:
            eis = const.tile([P, NT2, KZ], I32)
            nc.sync.dma_start(eis[:], eit[:, :, :])

            def load_x2tt(tt):
                # split into 4 DMAs so gather transfers interleave on the
                # shared DMA engines instead of queueing behind one 3.2us copy
                x2l = cx.tile([P, 2, KZ, P], BF16, tag="x2tt")
                for kq in range(4):
                    nc.sync.dma_start(x2l[:, :, 4 * kq:4 * (kq + 1), :],
                                      x2t[:, :, tt, 4 * kq:4 * (kq + 1), :])
                return x2l

            x2q = [load_x2tt(0), load_x2tt(1)]
            chunks = {}

            def stage1(t, kvg):
                t4 = t % CH
                if t4 == 0:
                    x1q = cp.tile([P, 2, CH * P], BF16, tag="x1q")
                    nc.sync.dma_start(x1q[:],
                                      x1qo[:, :, t * P:(t + CH) * P])
                    x1rc = cp.tile([P, CH, IFZ], F32, tag="x1rc")
                    nc.sync.dma_start(x1rc[:],
                                      x1rt[:, t:t + CH, :])
                    outw = cp.tile([P, CH, IFZ], F32, tag="outw")
                    chunks[t // CH] = (x1q, x1rc, outw)
                x1q, _, _ = chunks[t // CH]

                # ---- prefetch
                x2tt = x2q[t % 2]
                if t + 2 < NT2:
                    x2q[t % 2] = load_x2tt(t + 2)

                # ---- q/gate matmuls
                qg = cps.tile([P, 2 * HF], F32, tag="qg")
                for c in range(2):
                    nc.tensor.matmul(qg[:], x1q[:, c, t4 * P:(t4 + 1) * P],
                                     wqgb[:, c, :], start=(c == 0),
                                     stop=(c == 1))

                # ---- bias2 pre: coll[n, k, 0:8]=x2@(g*Wb), [n,k,8]=mean
                coll = cps.tile([P, KZ, 16], F32, tag="coll")
                for k in range(KZ):
                    for c in range(2):
                        nc.tensor.matmul(coll[:, k, :], x2tt[:, c, k, :],
                                         wbb[:, c, :], start=(c == 0),
                                         stop=(c == 1))

                # ---- sum of squares via ones-stationary matmul
                x2sq = cx1.tile([P, 2, KZ, P], BF16, tag="x2sq")
                nc.scalar.activation(x2sq[:], x2tt[:], AF.Square)
                # chunk q (n-group) -> psum row {0,32}[q%2], bank half q//2
                ssT = cps1.tile([33, 2 * 2 * HF], F32, tag="ssT")
                x2v = x2sq[:].rearrange("p c k n -> p c n k")
                for q in range(4):
                    r, b = 32 * (q % 2), 2 * HF * (q // 2)
                    for c in range(2):
                        nc.tensor.matmul(
                            ssT[r:r + 1, b:b + 2 * HF],
                            ones1[:], x2v[:, c, 32 * q:32 * (q + 1), :],
                            start=(c == 0), stop=(c == 1))
                sst_sb = cp.tile([33, 2 * 2 * HF], F32, tag="sst_sb")
                nc.scalar.copy(sst_sb[0:1, :], ssT[0:1, :])
                nc.scalar.copy(sst_sb[32:33, :], ssT[32:33, :])
                ssq = cp.tile([P, KZ], F32, tag="ssq")
                for q in range(4):
                    r, b = 32 * (q % 2), 2 * HF * (q // 2)
                    nc.scalar.dma_start(
                        ssq[32 * q:32 * (q + 1), :]
                        .rearrange("p (o k) -> p o k", o=1),
                        sst_sb[r:r + 1, b:b + 2 * HF]
                        .rearrange("o (n k) -> o n k", n=32))

                # ---- RoPE(q) (reads qg PSUM f32)
                qh = cp.tile([P, AHZ, AFZ], BF16, tag="qh")
                cosb = sc_own[:, t, None, 0:AFZ].to_broadcast([P, AHZ, AFZ])
                ssinb = sc_own[:, t, None, AFZ:2 * AFZ].to_broadcast(
                    [P, AHZ, AFZ])
                qv = qg[:, 0:HF].rearrange("p (h f) -> p h f", h=AHZ)
                qrot = qg[:, 0:HF].rearrange("p (h two g) -> p h two g",
                                             h=AHZ, two=2)
                tq1 = cp.tile([P, AHZ, AFZ], BF16, tag="tq1")
                nc.vector.tensor_tensor(tq1[:], qv, cosb, op=OP.mult)
                tq2 = cp.tile([P, AHZ, 2, AFZ // 2], BF16, tag="tq2")
                ss2 = ssinb.rearrange("p h (two g) -> p h two g", two=2)
                nc.vector.tensor_tensor(tq2[:, :, 0, :], qrot[:, :, 1, :],
                                        ss2[:, :, 0, :], op=OP.mult)
                nc.vector.tensor_tensor(tq2[:, :, 1, :], qrot[:, :, 0, :],
                                        ss2[:, :, 1, :], op=OP.mult)
                nc.vector.tensor_tensor(
                    qh[:], tq1[:],
                    tq2[:].rearrange("p h two g -> p h (two g)"), op=OP.add)

                # ---- gate = sigmoid(x) = 0.5*tanh(0.5x)+0.5
                xg = cp.tile([P, HF], F32, tag="xg")
                nc.vector.tensor_tensor(xg[:], qg[:, HF:2 * HF], bg_r,
                                        op=OP.add)
                th = cp.tile([P, HF], BF16, tag="th")
                nc.scalar.activation(th[:], xg[:], AF.Tanh, scale=0.5)
                gate = cp.tile([P, HF], BF16, tag="gate")
                nc.vector.tensor_scalar(gate[:], th[:], 0.5, 0.5,
                                        op0=OP.mult, op1=OP.add)

                # ---- scores
                kview = kvg[:, :, 0:HF].rearrange("p k (h f) -> p k h f",
                                                  h=AHZ)
                qb = qh[:, None, :, :].to_broadcast([P, KZ, AHZ, AFZ])
                prod = cd.tile([P, KZ, AHZ, AFZ], BF16, tag="prod")
                nc.vector.tensor_tensor(prod[:], kview, qb, op=OP.mult)
                # f-sum via bf16 TT halving tree
                sA = cd.tile([P, KZ, AHZ, 16], BF16, tag="sA")
                nc.vector.tensor_tensor(sA[:], prod[:, :, :, 0:16],
                                        prod[:, :, :, 16:32], op=OP.add)
                sB = cd.tile([P, KZ, AHZ, 8], BF16, tag="sB")
                nc.vector.tensor_tensor(sB[:], sA[:, :, :, 0:8],
                                        sA[:, :, :, 8:16], op=OP.add)
                sC = cd.tile([P, KZ, AHZ, 4], BF16, tag="sC")
                nc.vector.tensor_tensor(sC[:], sB[:, :, :, 0:4],
                                        sB[:, :, :, 4:8], op=OP.add)
                sD = cd.tile([P, KZ, AHZ, 2], BF16, tag="sD")
                nc.vector.tensor_tensor(sD[:], sC[:, :, :, 0:2],
                                        sC[:, :, :, 2:4], op=OP.add)
                sco = cp.tile([P, KZ, AHZ], BF16, tag="sco")
                nc.vector.tensor_tensor(sco[:], sD[:, :, :, 0],
                                        sD[:, :, :, 1], op=OP.add)
                sco2 = cp.tile([P, KZ, AHZ], F32, tag="sco2")

                # ---- bias2 terms
                mu = cp.tile([P, KZ], F32, tag="mu")
                nc.vector.tensor_scalar_mul(mu[:], coll[:, :, 8], 1.0)
                msq = cp.tile([P, KZ], F32, tag="msq")
                nc.vector.tensor_tensor(msq[:], mu[:], mu[:], op=OP.mult)
                var = cp.tile([P, KZ], F32, tag="var")
                nc.vector.scalar_tensor_tensor(var[:], ssq[:], 1.0 / IFZ,
                                               msq[:], op0=OP.mult,
                                               op1=OP.subtract)
                nc.vector.tensor_scalar(var[:], var[:], EPS, None, op0=OP.add)
                rstd = _newton_rsqrt(nc, cp, var[:], KZ, "rsb")
                t1b = cp.tile([P, KZ, AHZ], F32, tag="t1b")
                nc.vector.tensor_tensor(
                    t1b[:], coll[:, :, 8:9].to_broadcast([P, KZ, AHZ]),
                    sg_r[:, None, :].to_broadcast([P, KZ, AHZ]), op=OP.mult)
                t2b = cp.tile([P, KZ, AHZ], F32, tag="t2b")
                nc.vector.tensor_tensor(t2b[:], coll[:, :, 0:AHZ], t1b[:],
                                        op=OP.subtract)
                nc.vector.tensor_tensor(
                    t2b[:], t2b[:],
                    rstd[:, :, None].to_broadcast([P, KZ, AHZ]), op=OP.mult)
                nc.vector.tensor_tensor(sco2[:], sco[:], t2b[:], op=OP.add)
                nc.vector.tensor_tensor(
                    sco2[:], sco2[:],
                    tb_r[:, None, :].to_broadcast([P, KZ, AHZ]), op=OP.add)

                # ---- softmax exp on a broadcast view (weighted-V gets 2x)
                ee = cp.tile([P, KZ, AHZ, AFZ], BF16, tag="ee")
                nc.scalar.activation(
                    ee[:],
                    sco2[:, :, :, None].to_broadcast([P, KZ, AHZ, AFZ]),
                    AF.Exp)
                return dict(t=t, kvg=kvg, ee=ee, gate=gate)

            def stage2(s):
                t = s["t"]
                t4 = t % CH
                kvg, ee, gate = s["kvg"], s["ee"], s["gate"]
                _, x1rc, outw = chunks[t // CH]

                rsum = cp.tile([P, AHZ], F32, tag="rsum")
                nc.vector.tensor_reduce(rsum[:],
                                        ee[:, :, :, 0].rearrange(
                                            "p k h -> p h k"),
                                        axis=mybir.AxisListType.X, op=OP.add)
                rinv = cp.tile([P, AHZ], F32, tag="rinv")
                nc.vector.reciprocal(rinv[:], rsum[:])

                # ---- weighted V
                vview = kvg[:, :, HF:2 * HF].rearrange(
                    "p k (h f) -> p k h f", h=AHZ)
                wvt = cd.tile([P, KZ, AHZ, AFZ], BF16, tag="wvt")
                nc.vector.tensor_tensor(wvt[:], vview, ee[:], op=OP.mult)
                a1 = cd.tile([P, 8, AHZ, AFZ], BF16, tag="a1")
                nc.vector.tensor_tensor(a1[:], wvt[:, 0:8], wvt[:, 8:16],
                                        op=OP.add)
                a2 = cd.tile([P, 4, AHZ, AFZ], BF16, tag="a2")
                nc.vector.tensor_tensor(a2[:], a1[:, 0:4], a1[:, 4:8],
                                        op=OP.add)
                a3 = cd.tile([P, 2, AHZ, AFZ], BF16, tag="a3")
                nc.vector.tensor_tensor(a3[:], a2[:, 0:2], a2[:, 2:4],
                                        op=OP.add)
                au = cp.tile([P, AHZ, AFZ], BF16, tag="au")
                nc.vector.tensor_tensor(au[:], a3[:, 0], a3[:, 1], op=OP.add)

                # ---- att = au * gate * rinv
                gsc = cp.tile([P, AHZ, AFZ], BF16, tag="gsc")
                nc.vector.tensor_tensor(
                    gsc[:], gate[:].rearrange("p (h f) -> p h f", h=AHZ),
                    rinv[:, :, None].to_broadcast([P, AHZ, AFZ]), op=OP.mult)
                att = cp.tile([P, HF], BF16, tag="att")
                nc.vector.tensor_tensor(
                    att[:].rearrange("p (h f) -> p h f", h=AHZ), au[:],
                    gsc[:], op=OP.mult)

                # ---- back matmul
                attT = cp.tile([P, 2, P], BF16, tag="attT")
                nc.sync.dma_start_transpose(attT[:], att[:])
                bout = cps.tile([P, IFZ], F32, tag="bout")
                for c in range(2):
                    nc.tensor.matmul(bout[:], attT[:, c, :], wbackb[:, c, :],
                                     start=(c == 0), stop=(c == 1))

                # ---- residual + final layernorm
                res = cp.tile([P, IFZ], F32, tag="res")
                nc.vector.scalar_tensor_tensor(res[:], x1rc[:, t4, :],
                                               math.sqrt(2.0), bout[:],
                                               op0=OP.mult, op1=OP.add)
                nc.vector.tensor_tensor(res[:], res[:], bback_r, op=OP.add)
                smean = cp.tile([P, 1], F32, tag="smean")
                nc.vector.tensor_reduce(smean[:], res[:],
                                        axis=mybir.AxisListType.X, op=OP.add)
                scr = cp.tile([P, IFZ], BF16, tag="scr")
                ssf = cp.tile([P, 1], F32, tag="ssf")
                nc.scalar.activation(scr[:], res[:], AF.Square,
                                     accum_out=ssf[:])
                meanf = cp.tile([P, 1], F32, tag="meanf")
                nc.vector.tensor_scalar_mul(meanf[:], smean[:], 1.0 / IFZ)
                msqf = cp.tile([P, 1], F32, tag="msqf")
                nc.vector.tensor_tensor(msqf[:], meanf[:], meanf[:],
                                        op=OP.mult)
                varf = cp.tile([P, 1], F32, tag="varf")
                nc.vector.scalar_tensor_tensor(varf[:], ssf[:], 1.0 / IFZ,
                                               msqf[:], op0=OP.mult,
                                               op1=OP.subtract)
                nc.vector.tensor_scalar(varf[:], varf[:], EPS, None,
                                        op0=OP.add)
                rstdf = _newton_rsqrt(nc, cp, varf[:], 1, "rsf")
                nbias = cp.tile([P, 1], F32, tag="nbias")
                nc.vector.scalar_tensor_tensor(nbias[:], meanf[:], -1.0,
                                               rstdf[:], op0=OP.mult,
                                               op1=OP.mult)
                xn = cp.tile([P, IFZ], F32, tag="xn")
                nc.scalar.activation(xn[:], res[:], AF.Identity,
                                     scale=rstdf[:], bias=nbias[:])
                nc.vector.tensor_tensor(outw[:, t4, :], xn[:], lng_r,
                                        op=OP.mult)
                nc.vector.tensor_tensor(outw[:, t4, :], outw[:, t4, :],
                                        lnb_r, op=OP.add)
                if t4 == CH - 1:
                    nc.sync.dma_start(
                        out[(t - t4) * P:(t + 1) * P, :]
                        .rearrange("(j p) f -> p j f", p=P), outw[:])

            prev = None
            kvq = {}
            for t in range(NT2):
                if t % 2 == 0:
                    kvg2 = cg.tile([P, 2, KZ, 2 * HF], BF16, tag="kvg")
                    for d in range(2):
                        for j in range(KZ):
                            nc.gpsimd.indirect_dma_start(
                                out=kvg2[:, d, j, :], out_offset=None,
                                in_=Tt[:],
                                in_offset=IndirectOffsetOnAxis(
                                    ap=eis[:, t + d, j:j + 1], axis=0))
                    kvq[t] = kvg2[:, 0]
                    kvq[t + 1] = kvg2[:, 1]
                cur = stage1(t, kvq.pop(t))
                if prev is not None:
                    stage2(prev)
                prev = cur
            stage2(prev)

    nc.compile()
    return nc


_NC_CACHE = {}


def _get_nc(n_pad=NP, n_shard=NSH, n_cores=N_CORES):
    key = (n_pad, n_shard, n_cores)
    if key not in _NC_CACHE:
        _NC_CACHE[key] = build_nc(n_cores)
    return _NC_CACHE[key]


def _red(x):
    return (x - 2 * math.pi * np.round(x / (2 * math.pi))).astype(np.float32)


def _pos64(pos):
    """[cos-args | signed-sin-args]: Sin of this gives [cos | ssin] where
    ssin[f<16] = -sin, ssin[f>=16] = +sin."""
    n = pos.shape[0]
    o = np.zeros((n, 2 * AFZ), np.float32)
    o[:, 0:AFZ] = _red(pos + math.pi / 2)
    o[:, AFZ:AFZ + 16] = _red(pos[:, 0:16] + math.pi)
    o[:, AFZ + 16:2 * AFZ] = _red(pos[:, 16:32])
    return o


def make_in_maps(x_1, x_2, pos_emb, edge_index, Wq, Wk, Wv, Wb, bln_g, bln_b,
                 Wg, bg, Wback, bback, ln1_g, ln1_b, n_cores=N_CORES):
    x_1 = np.asarray(x_1, np.float32)
    x_2 = np.asarray(x_2, np.float32)
    pos_emb = np.asarray(pos_emb, np.float32)
    edge_index = np.asarray(edge_index).astype(np.int32)
    n = x_1.shape[0]
    n_shard = n // n_cores     # 2500

    # global tensors (shared by all cores)
    x1p = np.zeros((NP, IFZ), np.float32)
    x1p[:n] = x_1
    x1t = np.ascontiguousarray(
        x1p.T.reshape(2, P, NP), dtype=BF)           # x1t[c, f, n]
    p64 = np.zeros((NP, 2 * AFZ), np.float32)
    p64[:n] = _pos64(pos_emb)
    posf = np.ascontiguousarray(
        p64.reshape(NT1, P, 2 * AFZ).transpose(1, 0, 2))  # [p, t, 64]

    s = 1.0 / math.sqrt(AFZ)
    wkv = np.concatenate([np.asarray(Wk), np.asarray(Wv)], axis=1) \
        .reshape(2, P, 2 * HF).astype(BF)
    wkv = np.ascontiguousarray(wkv.transpose(1, 0, 2))   # [f, c, 512]
    wqg = np.concatenate([np.asarray(Wq) * s, np.asarray(Wg)], axis=1) \
        .reshape(2, P, 2 * HF).astype(BF)
    wqg = np.ascontiguousarray(wqg.transpose(1, 0, 2))
    wb = np.zeros((IFZ, 16), np.float32)
    wb[:, 0:AHZ] = np.asarray(bln_g)[:, None] * np.asarray(Wb)
    wb[:, AHZ] = 1.0 / IFZ
    wb16 = np.ascontiguousarray(
        wb.reshape(2, P, 16).astype(BF).transpose(1, 0, 2))
    wbk = np.asarray(Wback).reshape(2, P, IFZ).astype(BF)
    wback = np.ascontiguousarray(wbk.transpose(1, 0, 2))
    vecs = np.zeros((1, 5 * IFZ + 16), np.float32)
    vecs[0, 0:IFZ] = np.asarray(bg)
    vecs[0, IFZ:2 * IFZ] = np.asarray(ln1_g)
    vecs[0, 2 * IFZ:3 * IFZ] = np.asarray(ln1_b)
    vecs[0, 3 * IFZ:4 * IFZ] = np.asarray(bback)
    vecs[0, 5 * IFZ:5 * IFZ + 8] = np.asarray(bln_g) @ np.asarray(Wb)
    vecs[0, 5 * IFZ + 8:5 * IFZ + 16] = np.asarray(bln_b) @ np.asarray(Wb)

    common = dict(x1t=x1t, posf=posf, wkv=wkv, wqg=wqg, wb16=wb16,
                  wback=wback, vecs=vecs)

    in_maps = []
    for c in range(n_cores):
        lo = c * n_shard
        # poso [p, t, 64]
        po = np.zeros((NSH, 2 * AFZ), np.float32)
        po[:n_shard] = _pos64(pos_emb[lo:lo + n_shard])
        poso = np.ascontiguousarray(
            po.reshape(NT2, P, 2 * AFZ).transpose(1, 0, 2))
        # x2t [f, c2, t, k, m]
        x2s = np.zeros((NSH, KZ, IFZ), np.float32)
        x2s[:n_shard] = x_2[lo:lo + n_shard]
        # [t, m, k, c2, f] -> [f, c2, t, k, m]
        x2r = x2s.reshape(NT2, P, KZ, 2, P).astype(BF)
        x2tc = np.ascontiguousarray(x2r.transpose(4, 3, 0, 2, 1))
        # eit [p, t, k]
        ei = np.zeros((NSH, KZ), np.int32)
        ei[:n_shard] = edge_index[lo:lo + n_shard]
        eit = np.ascontiguousarray(
            ei.reshape(NT2, P, KZ).transpose(1, 0, 2))
        # x1rt [p, t, 256]
        x1r = np.zeros((NSH, IFZ), np.float32)
        x1r[:n_shard] = x_1[lo:lo + n_shard]
        x1rt = np.ascontiguousarray(
            x1r.reshape(NT2, P, IFZ).transpose(1, 0, 2))
        # x1qo [f, c2, m] (own shard, feature-major)
        x1qo = np.ascontiguousarray(
            x1r.T.reshape(2, P, NSH).transpose(1, 0, 2).astype(BF))
        m = dict(common)
        m.update(poso=poso, x2t=x2tc, eit=eit, x1rt=x1rt, x1qo=x1qo)
        in_maps.append(m)
    return in_maps, NP, n_shard


def kernel(**inputs):
    x_1 = np.asarray(inputs["x_1"], np.float32)
    n = x_1.shape[0]
    n_cores = N_CORES
    n_shard = n // n_cores
    in_maps, _, _ = make_in_maps(**inputs)
    nc = _get_nc(NP, NSH, n_cores)
    res = run_bass_kernel_spmd(nc, in_maps, core_ids=list(range(n_cores)),
                               trace=False)
    out = np.concatenate(
        [res.results[c]["out"][:n_shard] for c in range(n_cores)], axis=0)
    return out[:n].astype(np.float32)
